# revision 1
# baseline (speedup 1.0000x reference)
"""Bass/Trainium2 kernel for nn_Attention_84688165142614 (additive attention).

Computes, for full inputs (B=32, S=2048, EH=512, DH=512):
    enc    = enc_output.transpose(1, 0, 2)                  # [B, S, 2EH]
    energy = tanh(enc @ w_enc + (h @ w_dec) + attn_b)       # [B, S, DH]
    att    = energy @ v_w                                   # [B, S]
    att    = where(mask == 0, -1e10, att)
    out    = softmax(att, axis=1)

Strategy: data-parallel over batch across 8 NeuronCores (4 batches/core).
Host-side sharding lays out each core's enc shard feature-major
([b, e, s]) so the contraction dim (e) lands on SBUF partitions with
fully-contiguous DMA loads. The big matmul runs in bf16 (cast during the
SWDGE DMA; fp32 matmul costs 4 cycles/row on the PE vs 1 for bf16) with
fp32 PSUM accumulation; everything else stays fp32. Softmax skips the
max-subtraction pass (logits are bounded by sum|v| ~ 8; masked entries
reach exp() as ~-1e10 and underflow to exactly 0), so the denominator is
a single Exp+accum pass plus a ones-matmul partition reduction.
"""

import numpy as np
from contextlib import ExitStack

import concourse.bass as bass
import concourse.tile as tile
from concourse import bacc, mybir
from concourse.bass_utils import run_bass_kernel_spmd

# Problem shape (hardcoded; kernel.py must be self-contained).
B, S, E2, DH = 32, 2048, 1024, 512
N_CORES = 8
BC = B // N_CORES        # batches per core = 4
P = 128                  # SBUF partitions
EC = E2 // P             # enc-feature chunks = 8
ST = S // P              # s tiles = 16
D = DH                   # 512
KC = DH // P             # dec-feature chunks = 4
GRP = 4                  # s-tiles per psum group (4 groups of 4)
NG = ST // GRP

f32 = mybir.dt.float32
bf16 = mybir.dt.bfloat16
i32 = mybir.dt.int32
AF = mybir.ActivationFunctionType
ALU = mybir.AluOpType

NEG_BIG = -1.0e10

_NC_CACHE = None


def _emit(ctx, tc, nc, enc_t, h_t, mask_t, w_dec, w_enc, attn_b, v_w, sel_in, out):
    const = ctx.enter_context(tc.tile_pool(name="const", bufs=1))
    spsum = ctx.enter_context(tc.tile_pool(name="spsum", bufs=1, space="PSUM"))
    mpsum = ctx.enter_context(tc.tile_pool(name="mpsum", bufs=7, space="PSUM"))
    encp = ctx.enter_context(tc.tile_pool(name="encp", bufs=20))
    tmpp = ctx.enter_context(tc.tile_pool(name="tmpp", bufs=3))
    thp = ctx.enter_context(tc.tile_pool(name="thp", bufs=3))
    scrp = ctx.enter_context(tc.tile_pool(name="scrp", bufs=2))
    attp = ctx.enter_context(tc.tile_pool(name="attp", bufs=2))
    epip = ctx.enter_context(tc.tile_pool(name="epip", bufs=10))

    # ---- phase 0: small loads split across the two HWDGE queues ----
    wq = const.tile([P, EC * D], bf16)
    nc.sync.dma_start(out=wq[:], in_=w_enc[:])
    sel = const.tile([BC, BC * P], f32)
    nc.scalar.dma_start(out=sel[:], in_=sel_in[:])
    hT_sb = const.tile([P, KC * BC], f32)
    nc.sync.dma_start(out=hT_sb[:], in_=h_t[:])
    wdec_sb = const.tile([P, KC * D], f32)
    nc.sync.dma_start(out=wdec_sb[:], in_=w_dec[:])
    b_sb = const.tile([1, D], f32)
    nc.scalar.dma_start(out=b_sb[:], in_=attn_b[:])
    v_row = const.tile([1, D], f32)
    nc.scalar.dma_start(out=v_row[:], in_=v_w[:])
    mask_sb = const.tile([P, BC * ST], i32)
    nc.sync.dma_start(out=mask_sb[:], in_=mask_t[:])

    ones_row = const.tile([1, P], f32)      # [K=1, M<=128] stationary for bcasts
    nc.vector.memset(ones_row[:], 1.0)
    ones_mat = const.tile([P, P], f32)      # all-ones stationary: partition sums
    nc.vector.memset(ones_mat[:], 1.0)

    # mask additive term for all batches: (m - 1) * 1e10
    maddall = const.tile([P, BC * ST], f32)
    nc.vector.tensor_copy(maddall[:], mask_sb[:])
    nc.vector.tensor_scalar(
        out=maddall[:], in0=maddall[:], scalar1=-NEG_BIG, scalar2=NEG_BIG,
        op0=ALU.mult, op1=ALU.add,
    )

    # ---- phase 1: batch-0 enc loads queue on gpsimd (SWDGE, casting) ----
    enc_tiles = {}
    for ec in range(EC):
        t = encp.tile([P, S], bf16, tag="enc", name=f"enc_0_{ec}")
        if ec <= 2:
            half = S // 2
            nc.gpsimd.dma_start(out=t[:, :half], in_=enc_t[0, ec, :, :half])
            nc.gpsimd.dma_start(out=t[:, half:], in_=enc_t[0, ec, :, half:])
        else:
            nc.gpsimd.dma_start(out=t[:], in_=enc_t[0, ec])
        enc_tiles[(0, ec)] = t

    # ---- phase 2: dec[b, :] = h[b] @ w_dec + attn_b; broadcasts ----
    dec_ps = spsum.tile([BC, D], f32, tag="sp")
    for kc in range(KC):
        nc.tensor.matmul(
            dec_ps[:],
            lhsT=hT_sb[:, kc * BC : (kc + 1) * BC],
            rhs=wdec_sb[:, kc * D : (kc + 1) * D],
            start=(kc == 0),
            stop=False,
        )
    nc.tensor.matmul(
        dec_ps[:], lhsT=ones_row[:, 0:BC], rhs=b_sb[:], start=False, stop=True
    )
    dec_rows = const.tile([BC, D], f32)
    nc.vector.tensor_copy(dec_rows[:], dec_ps[:])

    dec_bc = const.tile([P, BC * D], f32)
    for b in range(BC):
        ps = spsum.tile([P, D], f32, tag="sp", name=f"decb_{b}")
        nc.tensor.matmul(
            ps[:], lhsT=sel[:, b * P : (b + 1) * P], rhs=dec_rows[:],
            start=True, stop=True,
        )
        nc.vector.tensor_copy(dec_bc[:, b * D : (b + 1) * D], ps[:])
    v_ps = spsum.tile([P, D], f32, tag="sp")
    nc.tensor.matmul(v_ps[:], lhsT=ones_row[:], rhs=v_row[:], start=True, stop=True)
    v_sb = const.tile([P, D], f32)
    nc.vector.tensor_copy(v_sb[:], v_ps[:])

    # ---- main loop ----
    for b in range(BC):
        # prefetch next batch's enc tiles
        if b + 1 < BC:
            for ec in range(EC):
                t = encp.tile([P, S], bf16, tag="enc", name=f"enc_{b+1}_{ec}")
                nc.gpsimd.dma_start(out=t[:], in_=enc_t[b + 1, ec])
                enc_tiles[(b + 1, ec)] = t

        att = attp.tile([P, ST], f32, tag="att", name=f"att_{b}")
        expt = epip.tile([P, ST], f32, tag="expt", name=f"expt_{b}")
        partc = epip.tile([P, ST], f32, tag="part", name=f"part_{b}")
        sizes = [4, 4, 4, 2, 2] if b == BC - 1 else [4, 4, 4, 4]
        starts = [sum(sizes[:i]) for i in range(len(sizes))]
        # Batch 0 streams in while computing: accumulate in two half-passes
        # (chunks 0-3 spill to SBUF, then 4-7) so psum slots retire at the
        # chunk-arrival rate and the PE stays dense during the HBM fill.
        split_accum = b == 0
        for sg, gsz in enumerate(sizes):
            spills = {}
            if split_accum:
                psA = [
                    mpsum.tile([P, D], f32, tag="mm", name=f"mmA_{b}_{sg}_{j}")
                    for j in range(gsz)
                ]
                for ec in range(EC // 2):
                    for j in range(gsz):
                        st = starts[sg] + j
                        nc.tensor.matmul(
                            psA[j][:],
                            lhsT=enc_tiles[(b, ec)][:, st * P : (st + 1) * P],
                            rhs=wq[:, ec * D : (ec + 1) * D],
                            start=(ec == 0),
                            stop=(ec == EC // 2 - 1),
                        )
                for j in range(gsz):
                    sp = tmpp.tile([P, D], f32, tag="spill", name=f"sp_{b}_{sg}_{j}")
                    nc.scalar.copy(sp[:], psA[j][:])
                    spills[j] = sp
            ec_lo = EC // 2 if split_accum else 0
            psums = [
                mpsum.tile([P, D], f32, tag="mm", name=f"mm_{b}_{sg}_{j}")
                for j in range(gsz)
            ]
            for ec in range(ec_lo, EC):
                for j in range(gsz):
                    st = starts[sg] + j
                    nc.tensor.matmul(
                        psums[j][:],
                        lhsT=enc_tiles[(b, ec)][:, st * P : (st + 1) * P],
                        rhs=wq[:, ec * D : (ec + 1) * D],
                        start=(ec == ec_lo),
                        stop=(ec == EC - 1),
                    )
            for j in range(gsz):
                st = starts[sg] + j
                if split_accum:
                    half = tmpp.tile([P, D], f32, tag="half", name=f"hf_{b}_{sg}_{j}")
                    nc.vector.tensor_add(half[:], psums[j][:], spills[j][:])
                    t_sb = tmpp.tile([P, D], f32, tag="tmp")
                    nc.vector.tensor_add(
                        t_sb[:], half[:], dec_bc[:, b * D : (b + 1) * D]
                    )
                else:
                    t_sb = tmpp.tile([P, D], f32, tag="tmp")
                    nc.vector.tensor_add(
                        t_sb[:], psums[j][:], dec_bc[:, b * D : (b + 1) * D]
                    )
                th = thp.tile([P, D], f32, tag="th")
                nc.scalar.activation(th[:], t_sb[:], AF.Tanh)
                scr = scrp.tile([P, D], f32, tag="scr")
                nc.vector.affine_mul_reduce(
                    out=scr[:],
                    accum_out=att[:, st : st + 1],
                    in0=th[:],
                    in1=v_sb[:],
                    scale=1.0,
                    bias=0.0,
                )
                # exp(att + madd) fused: bias supplies the mask term
                nc.scalar.activation(
                    expt[:, st : st + 1], att[:, st : st + 1], AF.Exp,
                    bias=maddall[:, b * ST + st : b * ST + st + 1],
                    accum_out=partc[:, st : st + 1],
                )

        # ---- epilogue tail: total on all partitions, reciprocal, scale ----
        partial = epip.tile([P, 1], f32, tag="partial", name=f"partsum_{b}")
        nc.vector.tensor_reduce(partial[:], partc[:], mybir.AxisListType.X, ALU.add)
        tot_ps = spsum.tile([P, 1], f32, tag="sp", name=f"tot_{b}")
        nc.tensor.matmul(
            tot_ps[:], lhsT=ones_mat[:], rhs=partial[:], start=True, stop=True
        )
        r_pp = epip.tile([P, 1], f32, tag="rpp", name=f"rpp_{b}")
        nc.vector.reciprocal(r_pp[:], tot_ps[:])
        out_sb = epip.tile([P, ST], f32, tag="outsb", name=f"osb_{b}")
        nc.vector.tensor_scalar_mul(out_sb[:], expt[:], r_pp[:])
        nc.sync.dma_start(out=out[b], in_=out_sb[:])


def build_nc():
    global _NC_CACHE
    if _NC_CACHE is not None:
        return _NC_CACHE
    nc = bacc.Bacc("TRN2", target_bir_lowering=False, debug=False)
    enc_t = nc.dram_tensor("enc_t", [BC, EC, P, S], f32, kind="ExternalInput").ap()
    h_t = nc.dram_tensor("h_t", [P, KC * BC], f32, kind="ExternalInput").ap()
    mask_t = nc.dram_tensor("mask_t", [P, BC * ST], i32, kind="ExternalInput").ap()
    w_dec = nc.dram_tensor("w_dec", [P, KC * D], f32, kind="ExternalInput").ap()
    w_enc = nc.dram_tensor("w_enc", [P, EC * D], bf16, kind="ExternalInput").ap()
    attn_b = nc.dram_tensor("attn_b", [1, D], f32, kind="ExternalInput").ap()
    v_w = nc.dram_tensor("v_w", [1, D], f32, kind="ExternalInput").ap()
    sel_in = nc.dram_tensor("sel_in", [BC, BC * P], f32, kind="ExternalInput").ap()
    out = nc.dram_tensor("out", [BC, P, ST], f32, kind="ExternalOutput").ap()

    with tile.TileContext(nc) as tc:
        with ExitStack() as ctx:
            _emit(ctx, tc, nc, enc_t, h_t, mask_t, w_dec, w_enc, attn_b, v_w, sel_in, out)
    nc.compile()
    _NC_CACHE = nc
    return nc


def shard_inputs(inputs):
    h = np.asarray(inputs["h"], dtype=np.float32)
    enc = np.asarray(inputs["enc_output"], dtype=np.float32)
    mask = np.asarray(inputs["mask"], dtype=np.int32)
    attn_w = np.asarray(inputs["attn_w"], dtype=np.float32)
    attn_b = np.asarray(inputs["attn_b"], dtype=np.float32)
    v_w = np.asarray(inputs["v_w"], dtype=np.float32)

    # w_dec [DH, D] -> [P, KC*D] with free index (kc, d)
    w_dec = np.ascontiguousarray(
        attn_w[:DH].reshape(KC, P, D).transpose(1, 0, 2).reshape(P, KC * D)
    )
    # w_enc [E2, D] -> [P, EC*D] with free index (ec, d), pre-cast to bf16
    import ml_dtypes
    w_enc = np.ascontiguousarray(
        attn_w[DH:].reshape(EC, P, D).transpose(1, 0, 2).reshape(P, EC * D)
    ).astype(ml_dtypes.bfloat16)
    b_row = np.ascontiguousarray(attn_b).reshape(1, D)
    v_row = np.ascontiguousarray(v_w).reshape(1, D)
    sel_np = np.zeros((BC, BC * P), dtype=np.float32)
    for b in range(BC):
        sel_np[b, b * P : (b + 1) * P] = 1.0

    in_maps = []
    for c in range(N_CORES):
        bs = slice(BC * c, BC * (c + 1))
        enc_t = np.ascontiguousarray(enc[:, bs, :].transpose(1, 2, 0)).reshape(
            BC, EC, P, S
        )
        # h [BC, DH] -> [P, (kc, b)]
        h_t = np.ascontiguousarray(
            h[bs].T.reshape(KC, P, BC).transpose(1, 0, 2).reshape(P, KC * BC)
        )
        # mask [BC, S] -> [P, (b, t)]
        mask_t = np.ascontiguousarray(
            mask[bs].reshape(BC, ST, P).transpose(2, 0, 1).reshape(P, BC * ST)
        )
        in_maps.append(
            dict(
                enc_t=enc_t, h_t=h_t, mask_t=mask_t,
                w_dec=w_dec, w_enc=w_enc, attn_b=b_row, v_w=v_row, sel_in=sel_np,
            )
        )
    return in_maps


def run(inputs, trace=False):
    nc = build_nc()
    in_maps = shard_inputs(inputs)
    res = run_bass_kernel_spmd(nc, in_maps, list(range(N_CORES)), trace=trace)
    outs = [
        res.results[c]["out"].reshape(BC, P, ST).transpose(0, 2, 1).reshape(BC, S)
        for c in range(N_CORES)
    ]
    return np.concatenate(outs, axis=0).astype(np.float32), res


def kernel(**inputs) -> np.ndarray:
    out, _ = run(inputs, trace=False)
    return out



# revision 2
# speedup vs baseline: 1.7372x; 1.7372x over previous
"""Bass/Trainium2 kernel for nn_Attention_84688165142614 (additive attention).

Computes, for full inputs (B=32, S=2048, EH=512, DH=512):
    enc    = enc_output.transpose(1, 0, 2)                  # [B, S, 2EH]
    energy = tanh(enc @ w_enc + (h @ w_dec) + attn_b)       # [B, S, DH]
    att    = energy @ v_w                                   # [B, S]
    att    = where(mask == 0, -1e10, att)
    out    = softmax(att, axis=1)

Strategy: data-parallel over batch across 8 NeuronCores (4 batches/core),
plus mask-sparsity compaction. The mask is ~50% zeros and masked positions
produce exactly 0 in the reference output (exp(-1e10) underflows in f32),
so the host keeps only unmasked source positions per batch (gather),
pads each batch to a fixed multiple of 128 columns, transposes the kept
enc columns feature-major ([b, e, s_c]) and pre-casts to bf16. The device
computes energies/logits/softmax only for the compacted columns (pads are
killed with a -1e10 additive bias fused into the Exp), and the host
scatters the compacted probabilities back into a zero [B, S] output.
The big matmul runs in bf16 with fp32 PSUM accumulation; softmax skips
the max-subtraction pass (logits are bounded by sum|v| ~ 8).
"""

import numpy as np
from contextlib import ExitStack

import concourse.bass as bass
import concourse.tile as tile
from concourse import bacc, mybir
from concourse.bass_utils import run_bass_kernel_spmd

# Problem shape (hardcoded; kernel.py must be self-contained).
B, S, E2, DH = 32, 2048, 1024, 512
N_CORES = 8
BC = B // N_CORES        # batches per core = 4
P = 128                  # SBUF partitions
EC = E2 // P             # enc-feature chunks = 8
D = DH                   # 512
KC = DH // P             # dec-feature chunks = 4
NT_DEFAULT = 9           # compacted s-tiles per batch (1152 cols, ~5.7 sigma)

f32 = mybir.dt.float32
bf16 = mybir.dt.bfloat16
AF = mybir.ActivationFunctionType
ALU = mybir.AluOpType

NEG_BIG = -1.0e10

_NC_CACHE = {}


def _emit(ctx, tc, nc, nt, enc_t, h_t, madd_in, w_dec, w_enc, attn_b, v_w, sel_in, out):
    W = nt * P
    const = ctx.enter_context(tc.tile_pool(name="const", bufs=1))
    spsum = ctx.enter_context(tc.tile_pool(name="spsum", bufs=1, space="PSUM"))
    mpsum = ctx.enter_context(tc.tile_pool(name="mpsum", bufs=7, space="PSUM"))
    encp = ctx.enter_context(tc.tile_pool(name="encp", bufs=20))
    tmpp = ctx.enter_context(tc.tile_pool(name="tmpp", bufs=3))
    thp = ctx.enter_context(tc.tile_pool(name="thp", bufs=3))
    scrp = ctx.enter_context(tc.tile_pool(name="scrp", bufs=2))
    attp = ctx.enter_context(tc.tile_pool(name="attp", bufs=2))
    epip = ctx.enter_context(tc.tile_pool(name="epip", bufs=10))

    # ---- phase 0: small loads split across the two HWDGE queues ----
    wq = const.tile([P, EC * D], bf16)
    nc.sync.dma_start(out=wq[:], in_=w_enc[:])
    sel = const.tile([BC, BC * P], f32)
    nc.scalar.dma_start(out=sel[:], in_=sel_in[:])
    hT_sb = const.tile([P, KC * BC], f32)
    nc.sync.dma_start(out=hT_sb[:], in_=h_t[:])
    wdec_sb = const.tile([P, KC * D], f32)
    nc.sync.dma_start(out=wdec_sb[:], in_=w_dec[:])
    b_sb = const.tile([1, D], f32)
    nc.scalar.dma_start(out=b_sb[:], in_=attn_b[:])
    v_row = const.tile([1, D], f32)
    nc.scalar.dma_start(out=v_row[:], in_=v_w[:])
    madd_sb = const.tile([P, BC * nt], f32)
    nc.scalar.dma_start(out=madd_sb[:], in_=madd_in[:])

    ones_row = const.tile([1, P], f32)      # [K=1, M<=128] stationary for bcasts
    nc.vector.memset(ones_row[:], 1.0)
    ones_mat = const.tile([P, P], f32)      # all-ones stationary: partition sums
    nc.vector.memset(ones_mat[:], 1.0)

    # ---- phase 1: batch-0 enc loads, spread across DMA queues ----
    enc_tiles = {}
    for ec in range(EC):
        t = encp.tile([P, W], bf16, tag="enc", name=f"enc_0_{ec}")
        if ec < 2:
            half = W // 2
            eng = [nc.sync, nc.scalar][ec % 2]
            eng.dma_start(out=t[:, :half], in_=enc_t[0, ec, :, :half])
            eng.dma_start(out=t[:, half:], in_=enc_t[0, ec, :, half:])
        else:
            eng = [nc.sync, nc.scalar, nc.gpsimd][ec % 3]
            eng.dma_start(out=t[:], in_=enc_t[0, ec])
        enc_tiles[(0, ec)] = t

    # ---- phase 2: dec[b, :] = h[b] @ w_dec + attn_b; broadcasts ----
    dec_ps = spsum.tile([BC, D], f32, tag="sp")
    for kc in range(KC):
        nc.tensor.matmul(
            dec_ps[:],
            lhsT=hT_sb[:, kc * BC : (kc + 1) * BC],
            rhs=wdec_sb[:, kc * D : (kc + 1) * D],
            start=(kc == 0),
            stop=False,
        )
    nc.tensor.matmul(
        dec_ps[:], lhsT=ones_row[:, 0:BC], rhs=b_sb[:], start=False, stop=True
    )
    dec_rows = const.tile([BC, D], f32)
    nc.vector.tensor_copy(dec_rows[:], dec_ps[:])

    dec_bc = const.tile([P, BC * D], f32)
    for b in range(BC):
        ps = spsum.tile([P, D], f32, tag="sp", name=f"decb_{b}")
        nc.tensor.matmul(
            ps[:], lhsT=sel[:, b * P : (b + 1) * P], rhs=dec_rows[:],
            start=True, stop=True,
        )
        nc.vector.tensor_copy(dec_bc[:, b * D : (b + 1) * D], ps[:])
    v_ps = spsum.tile([P, D], f32, tag="sp")
    nc.tensor.matmul(v_ps[:], lhsT=ones_row[:], rhs=v_row[:], start=True, stop=True)
    v_sb = const.tile([P, D], f32)
    nc.vector.tensor_copy(v_sb[:], v_ps[:])

    sizes = [4] * (nt // 4)
    if nt % 4:
        sizes.append(nt % 4)
    starts = [sum(sizes[:i]) for i in range(len(sizes))]

    # ---- main loop ----
    for b in range(BC):
        # prefetch next batch's enc tiles
        if b + 1 < BC:
            for ec in range(EC):
                t = encp.tile([P, W], bf16, tag="enc", name=f"enc_{b+1}_{ec}")
                eng = [nc.sync, nc.scalar, nc.gpsimd][ec % 3]
                eng.dma_start(out=t[:], in_=enc_t[b + 1, ec])
                enc_tiles[(b + 1, ec)] = t

        att = attp.tile([P, nt], f32, tag="att", name=f"att_{b}")
        expt = epip.tile([P, nt], f32, tag="expt", name=f"expt_{b}")
        partc = epip.tile([P, nt], f32, tag="part", name=f"part_{b}")
        for sg, gsz in enumerate(sizes):
            psums = [
                mpsum.tile([P, D], f32, tag="mm", name=f"mm_{b}_{sg}_{j}")
                for j in range(gsz)
            ]
            for ec in range(EC):
                for j in range(gsz):
                    st = starts[sg] + j
                    nc.tensor.matmul(
                        psums[j][:],
                        lhsT=enc_tiles[(b, ec)][:, st * P : (st + 1) * P],
                        rhs=wq[:, ec * D : (ec + 1) * D],
                        start=(ec == 0),
                        stop=(ec == EC - 1),
                    )
            for j in range(gsz):
                st = starts[sg] + j
                t_sb = tmpp.tile([P, D], f32, tag="tmp")
                nc.vector.tensor_add(
                    t_sb[:], psums[j][:], dec_bc[:, b * D : (b + 1) * D]
                )
                th = thp.tile([P, D], f32, tag="th")
                nc.scalar.activation(th[:], t_sb[:], AF.Tanh)
                scr = scrp.tile([P, D], f32, tag="scr")
                nc.vector.affine_mul_reduce(
                    out=scr[:],
                    accum_out=att[:, st : st + 1],
                    in0=th[:],
                    in1=v_sb[:],
                    scale=1.0,
                    bias=0.0,
                )
                # exp(att + madd) fused: bias supplies the pad-kill term
                nc.scalar.activation(
                    expt[:, st : st + 1], att[:, st : st + 1], AF.Exp,
                    bias=madd_sb[:, b * nt + st : b * nt + st + 1],
                    accum_out=partc[:, st : st + 1],
                )

        # ---- epilogue tail: total on all partitions, reciprocal, scale ----
        partial = epip.tile([P, 1], f32, tag="partial", name=f"partsum_{b}")
        nc.vector.tensor_reduce(partial[:], partc[:], mybir.AxisListType.X, ALU.add)
        tot_ps = spsum.tile([P, 1], f32, tag="sp", name=f"tot_{b}")
        nc.tensor.matmul(
            tot_ps[:], lhsT=ones_mat[:], rhs=partial[:], start=True, stop=True
        )
        r_pp = epip.tile([P, 1], f32, tag="rpp", name=f"rpp_{b}")
        nc.vector.reciprocal(r_pp[:], tot_ps[:])
        out_sb = epip.tile([P, nt], f32, tag="outsb", name=f"osb_{b}")
        nc.vector.tensor_scalar_mul(out_sb[:], expt[:], r_pp[:])
        nc.sync.dma_start(out=out[b], in_=out_sb[:])


def build_nc(nt):
    if nt in _NC_CACHE:
        return _NC_CACHE[nt]
    W = nt * P
    nc = bacc.Bacc("TRN2", target_bir_lowering=False, debug=False)
    enc_t = nc.dram_tensor("enc_t", [BC, EC, P, W], bf16, kind="ExternalInput").ap()
    h_t = nc.dram_tensor("h_t", [P, KC * BC], f32, kind="ExternalInput").ap()
    madd = nc.dram_tensor("madd", [P, BC * nt], f32, kind="ExternalInput").ap()
    w_dec = nc.dram_tensor("w_dec", [P, KC * D], f32, kind="ExternalInput").ap()
    w_enc = nc.dram_tensor("w_enc", [P, EC * D], bf16, kind="ExternalInput").ap()
    attn_b = nc.dram_tensor("attn_b", [1, D], f32, kind="ExternalInput").ap()
    v_w = nc.dram_tensor("v_w", [1, D], f32, kind="ExternalInput").ap()
    sel_in = nc.dram_tensor("sel_in", [BC, BC * P], f32, kind="ExternalInput").ap()
    out = nc.dram_tensor("out", [BC, P, nt], f32, kind="ExternalOutput").ap()

    with tile.TileContext(nc) as tc:
        with ExitStack() as ctx:
            _emit(ctx, tc, nc, nt, enc_t, h_t, madd, w_dec, w_enc, attn_b, v_w,
                  sel_in, out)
    nc.compile()
    _NC_CACHE[nt] = nc
    return nc


def shard_inputs(inputs, nt):
    import ml_dtypes

    W = nt * P
    h = np.asarray(inputs["h"], dtype=np.float32)
    enc = np.asarray(inputs["enc_output"], dtype=np.float32)
    mask = np.asarray(inputs["mask"], dtype=np.int32)
    attn_w = np.asarray(inputs["attn_w"], dtype=np.float32)
    attn_b = np.asarray(inputs["attn_b"], dtype=np.float32)
    v_w = np.asarray(inputs["v_w"], dtype=np.float32)

    # w_dec [DH, D] -> [P, KC*D] with free index (kc, d)
    w_dec = np.ascontiguousarray(
        attn_w[:DH].reshape(KC, P, D).transpose(1, 0, 2).reshape(P, KC * D)
    )
    # w_enc [E2, D] -> [P, EC*D] with free index (ec, d), pre-cast to bf16
    w_enc = np.ascontiguousarray(
        attn_w[DH:].reshape(EC, P, D).transpose(1, 0, 2).reshape(P, EC * D)
    ).astype(ml_dtypes.bfloat16)
    b_row = np.ascontiguousarray(attn_b).reshape(1, D)
    v_row = np.ascontiguousarray(v_w).reshape(1, D)
    sel_np = np.zeros((BC, BC * P), dtype=np.float32)
    for b in range(BC):
        sel_np[b, b * P : (b + 1) * P] = 1.0

    kept = [np.nonzero(mask[gb])[0] for gb in range(B)]

    in_maps = []
    for c in range(N_CORES):
        enc_c = np.zeros((BC, EC, P, W), dtype=ml_dtypes.bfloat16)
        madd = np.full((P, BC * nt), NEG_BIG, dtype=np.float32)
        h_t = np.ascontiguousarray(
            h[BC * c : BC * (c + 1)]
            .T.reshape(KC, P, BC)
            .transpose(1, 0, 2)
            .reshape(P, KC * BC)
        )
        for b in range(BC):
            gb = BC * c + b
            idx = kept[gb]
            n = len(idx)
            # kept enc columns, feature-major: [2EH, n] -> [EC, P, n]
            cols = enc[idx, gb, :].T.astype(ml_dtypes.bfloat16)
            enc_c[b, :, :, :n] = cols.reshape(EC, P, n)
            # compact additive mask: 0 for real columns, -1e10 for pads
            flat = madd.reshape(P, BC, nt)
            m = np.zeros(W, dtype=np.float32)
            m[n:] = NEG_BIG
            flat[:, b, :] = m.reshape(nt, P).T
        in_maps.append(
            dict(
                enc_t=enc_c, h_t=h_t, madd=madd,
                w_dec=w_dec, w_enc=w_enc, attn_b=b_row, v_w=v_row, sel_in=sel_np,
            )
        )
    return in_maps, kept


def run(inputs, trace=False):
    mask = np.asarray(inputs["mask"], dtype=np.int32)
    counts = mask.sum(axis=1)
    nt = max(NT_DEFAULT, int(np.ceil(counts.max() / P)))
    nc = build_nc(nt)
    in_maps, kept = shard_inputs(inputs, nt)
    res = run_bass_kernel_spmd(nc, in_maps, list(range(N_CORES)), trace=trace)
    out_full = np.zeros((B, S), dtype=np.float32)
    for c in range(N_CORES):
        vals = res.results[c]["out"].reshape(BC, P, nt)
        for b in range(BC):
            gb = BC * c + b
            idx = kept[gb]
            flat = vals[b].T.reshape(nt * P)
            out_full[gb, idx] = flat[: len(idx)]
    return out_full, res


def kernel(**inputs) -> np.ndarray:
    out, _ = run(inputs, trace=False)
    return out
